# revision 3
# baseline (speedup 1.0000x reference)
"""Trainium2 Bass kernel for nn_DetectionLayer (refine + per-class NMS + top-100).

Collective-free SPMD (8 cores run the identical latency-bound program).
v3 pipeline:

  1. probs [5000, 81] loads as ONE dma (125 x 12.96KB descriptors -- big
     descriptors beat multi-queue round-robin, which shares the 16 SDMA
     engines fairly and makes every chunk finish late).
  2. Ladder threshold from per-partition top-8 score order statistics
     (DVE max8): one is_ge + one bf16 PE count matmul, full-population
     MINC; exact for the rungs near the cut (per-partition candidate
     count <= 5 there).
  3. Compaction: mask codes in [128, 40], DVE max8 pre-compacts to
     [128, 8], one relayout DMA to [16, 64], sparse_gather scans 1024
     elements instead of 5120.
  4. The compacted [16, 12] code table feeds the indirect gathers
     DIRECTLY as the offset AP (no [128, 2] relayout on the critical
     path); the f32 row-id relayout for tie-breaks rides the gpsimd
     queue behind the gathers.
  5. Gathers: joined rows (ROI|probs) + full 81-class delta blocks;
     class-specific delta via one-hot select after the argmax.
  6. Class-separation trick: pairwise stage uses y + 2*cls so the
     same-class mask disappears (verified exact).
  7. Replicated row matrices via SBUF->DRAM->broadcast-read DMAs (no
     fp32 PE matmuls, no PSUM copies).
  8. Order matrix on ACT+DVE: O = (2*sign(s_c' - s_c) + sign(gi_c -
     gi_c') > 0); suppression matrix split DVE-style (chunk0) and
     ACT-style via Relu-with-bias identities (chunk1).
  9. Greedy-NMS Jacobi fixpoint (2 iters, verified), rank one-hot
     scatter matmul.
"""

import numpy as np

import concourse.bacc as bacc
import concourse.bass as bass
import concourse.mybir as mybir
import concourse.tile as tile
from concourse.alu_op_type import AluOpType as ALU

F32 = mybir.dt.float32
BF16 = mybir.dt.bfloat16
I32 = mybir.dt.int32
U32 = mybir.dt.uint32

NCORES = 8
N = 5000
PA = 125
TA = N // PA                 # 40 rows per partition
NCLS = 81
NSLOT = 192
CH = 2
CHS = (128, 64)
NITER = 2
R = 100
NLAD = 32
MINC = 112.0 / 0.75          # full-population count target
MIN_CONF = 0.7
NMS_THR = 0.3

# const block column layout (c128 [128, 255])
C_IOTAD = 0        # 81: NCLS - arange(NCLS)
C_IOTAR = 81       # 100: arange(R)
C_SLOT = 181       # 2: compacted index at slot (p, k), 9999 for dead
C_LAD = 183        # 32: ladder thresholds (ascending)
C_POS = 215        # 40: row-id code 40p + t + 1
C_NCOL = 255


def _consts():
    c128 = np.zeros((128, C_NCOL), np.float32)
    c128[:, C_IOTAD:C_IOTAD + NCLS] = NCLS - np.arange(NCLS, dtype=np.float32)
    c128[:, C_IOTAR:C_IOTAR + R] = np.arange(R, dtype=np.float32)
    p = np.arange(128)
    c128[:, C_SLOT] = p // 8 + 16 * (p % 8)
    c1 = np.full(128, 9999.0, np.float32)
    c1[:64] = p[:64] // 4 + 16 * (p[:64] % 4 + 8)
    c128[:, C_SLOT + 1] = c1
    targets = np.minimum(144.0 * 1.1 ** np.arange(NLAD), 4999.0)
    lad = np.sort(((1.0 - targets / N) ** (1.0 / NCLS)).astype(np.float32))
    c128[:, C_LAD:C_LAD + NLAD] = lad
    c128[:, C_POS:C_POS + TA] = (p[:, None] * TA + np.arange(TA)[None, :] + 1)
    ident = np.eye(128, dtype=np.float32)
    return c128, ident


def build(nc: bass.Bass, tc: tile.TileContext, outs, ins):
    det = outs["det"]
    probs, deltas = ins["probs"], ins["deltas"]
    joined = ins["joined"]
    window = ins["window"]
    c128_np, ident_np = _consts()
    c128_d = nc.inline_tensor(c128_np, name="c_c128").ap()
    ident_d = nc.inline_tensor(ident_np, name="c_ident").ap()

    AX = mybir.AxisListType.X
    ACT = mybir.ActivationFunctionType

    with (
        tc.tile_pool(name="a", bufs=1) as pa,
        tc.tile_pool(name="b", bufs=1) as pb,
        tc.tile_pool(name="ps", bufs=1, space="PSUM") as pps,
        tc.tile_pool(name="dr", bufs=1, space="DRAM") as pdr,
    ):
        # ---------------- phase 0: probs first, consts behind ---------------
        probs_t = pa.tile([PA, TA, NCLS], F32)
        nc.sync.dma_start(probs_t[:].rearrange("p t c -> p (t c)"),
                          probs.rearrange("(p j) c -> p (j c)", p=PA))
        c128 = pb.tile([128, C_NCOL], F32)
        identity = pb.tile([128, 128], F32)
        nc.scalar.dma_start(c128[:], c128_d[:])
        nc.scalar.dma_start(identity[:], ident_d[:])
        winb = pb.tile([128, 4], F32)
        nc.scalar.dma_start(winb[:], window[:].broadcast_to((128, 4)))

        iotaDb = c128[:, C_IOTAD:C_IOTAD + NCLS]
        iotaRf = c128[:, C_IOTAR:C_IOTAR + R]
        slotid = c128[:, C_SLOT:C_SLOT + CH]
        ladb = c128[0:PA, C_LAD:C_LAD + NLAD]
        lad1 = c128[0:1, C_LAD:C_LAD + NLAD]
        posc = c128[:, C_POS:C_POS + TA]

        maxv = pa.tile([128, TA], F32)
        nc.vector.memset(maxv[:], -1.0)
        ones125 = pa.tile([PA, 1], BF16)
        ones1 = pa.tile([1, 128], F32)
        nc.vector.memset(ones125[:], 1.0)
        nc.vector.memset(ones1[:], 1.0)
        gj = pb.tile([128, CH, 4 + NCLS], F32)
        nc.vector.memset(gj[:], 0.0)
        gdall = pb.tile([128, CH, NCLS * 4], F32)
        nc.vector.memset(gdall[:], 0.0)
        gall = pb.tile([128, CH, 8], F32)   # y1 x1 y2 x2 a03 cls score rowid
        nc.vector.memset(gall[:], 0.0)
        gmat = pb.tile([128, CH, 5], F32)   # ys1 x1 ys2 x2 a03 (shifted ys)
        nc.vector.memset(gmat[:], 0.0)

        misc_ps = pps.tile([128, 512], F32, tag="misc")
        b128t_ps = misc_ps[:, 4:5]
        b128_ps = misc_ps[:, 5:6]
        tr1_ps = misc_ps[0:2, 6:134]
        tr2_ps = misc_ps[0:5, 134:262]
        out_ps = misc_ps[0:R, 264:272]
        cnt_ps = pps.tile([1, 8 * NLAD], F32, tag="cnt")

        # ---------------- phase 1: scores, ladder via top-8 -----------------
        tc_sz = TA // 4
        for kc in range(4):
            js = slice(kc * tc_sz, (kc + 1) * tc_sz)
            nc.vector.tensor_reduce(maxv[0:PA, js], probs_t[:, js, :], AX,
                                    ALU.max)
        m8s = pa.tile([PA, 8], F32)
        nc.vector.max(m8s[:], maxv[0:PA, :])
        ind8 = pa.tile([PA, 8, NLAD], BF16)
        nc.vector.tensor_tensor(
            ind8[:], m8s[:].unsqueeze(2).broadcast_to((PA, 8, NLAD)),
            ladb.unsqueeze(1).broadcast_to((PA, 8, NLAD)), ALU.is_ge)
        nc.tensor.matmul(cnt_ps[:], ones125[:],
                         ind8[:].rearrange("p t r -> p (t r)"),
                         start=True, stop=True)
        cnt32 = pa.tile([1, NLAD], F32)
        nc.vector.tensor_reduce(cnt32[:],
                                cnt_ps[:].rearrange("p (t r) -> p r t", t=8),
                                AX, ALU.add)
        ltv = pa.tile([1, NLAD], F32)
        nc.vector.scalar_tensor_tensor(ltv[:], cnt32[:], MINC, lad1,
                                       op0=ALU.is_ge, op1=ALU.mult)
        tstar = pa.tile([1, 1], F32)
        nc.vector.tensor_reduce(tstar[:], ltv[:], AX, ALU.max)
        nc.tensor.matmul(b128t_ps, ones1[:], tstar[:], start=True, stop=True)

        # mask codes + per-partition top-8 pre-compaction
        mi = pb.tile([128, TA], F32)
        nc.vector.scalar_tensor_tensor(mi[:], maxv[:], b128t_ps, posc,
                                       op0=ALU.is_ge, op1=ALU.mult)
        nc.vector.tensor_scalar_add(mi[:], mi[:], -1.0)
        m8mi = pb.tile([128, 8], F32)
        nc.vector.max(m8mi[:], mi[:])
        s16c = pb.tile([16, 64], F32)
        nc.sync.dma_start(s16c[:], m8mi[:])
        sgout = pb.tile([16, NSLOT // 16], F32)
        nf = pb.tile([1, 1], U32)
        nc.gpsimd.sparse_gather(sgout[:], s16c[:], num_found=nf[:])

        # ---------------- phase 2: gathers straight off the code table ------
        sgc = pb.tile([16, NSLOT // 16], F32)
        nc.vector.tensor_scalar(sgc[:], sgout[:], 0.0, float(N - 1),
                                op0=ALU.max, op1=ALU.min)
        sgi = pb.tile([16, NSLOT // 16], I32)
        nc.vector.tensor_copy(sgi[:], sgc[:])
        rfi = pb.tile([128, CH], I32)
        nc.vector.memset(rfi[:], 0)
        nc.sync.dma_start(rfi[:, 0:1], sgi[:, 0:8])
        nc.scalar.dma_start(rfi[0:64, 1:2], sgi[:, 8:12])
        soff = (rfi[0:128, 0:1], rfi[0:64, 1:2])
        deltas_blk = deltas.rearrange("r c e -> r (c e)")
        # both joined gathers first (argmax needs them fused), deltas after
        nc.gpsimd.indirect_dma_start(
            out=gj[0:128, 0, :], out_offset=None, in_=joined,
            in_offset=bass.IndirectOffsetOnAxis(ap=soff[0], axis=0))
        nc.gpsimd.indirect_dma_start(
            out=gj[0:64, 1, :], out_offset=None, in_=joined,
            in_offset=bass.IndirectOffsetOnAxis(ap=soff[1], axis=0))
        nc.gpsimd.indirect_dma_start(
            out=gdall[0:128, 0, :], out_offset=None, in_=deltas_blk,
            in_offset=bass.IndirectOffsetOnAxis(ap=soff[0], axis=0))
        nc.gpsimd.indirect_dma_start(
            out=gdall[0:64, 1, :], out_offset=None, in_=deltas_blk,
            in_offset=bass.IndirectOffsetOnAxis(ap=soff[1], axis=0))
        # f32 row-ids into slot layout (tie-breaks) -- on the HWDGE queues so
        # their 4-byte descriptors don't clog the gpsimd SWDGE ring ahead of
        # the gathers
        rfc = pb.tile([128, CH], F32)
        nc.vector.memset(rfc[:], 0.0)
        nc.sync.dma_start(rfc[:, 0:1], sgc[:, 0:8])
        nc.scalar.dma_start(rfc[0:64, 1:2], sgc[:, 8:12])

        nf_f = pb.tile([1, 1], F32)
        nc.vector.tensor_copy(nf_f[:], nf[:])
        nc.tensor.matmul(b128_ps, ones1[:], nf_f[:], start=True, stop=True)
        q2 = pb.tile([128, CH], F32)
        nc.vector.tensor_scalar(q2[:], slotid, b128_ps, None, op0=ALU.is_lt)

        # ---------------- phase 3: per-candidate compute ---------------------
        gr2v = gj[:][:, :, 0:4]
        gp2v = gj[:][:, :, 4:4 + NCLS]
        maxc2 = pb.tile([128, CH], F32)
        nc.vector.tensor_reduce(maxc2[:], gp2v, AX, ALU.max)
        onehot2 = pb.tile([128, CH, NCLS], F32)
        nc.vector.tensor_tensor(
            onehot2[:], gp2v,
            maxc2[:].unsqueeze(2).broadcast_to((128, CH, NCLS)), ALU.is_equal)
        prodc2 = pb.tile([128, CH, NCLS], F32)
        nc.vector.tensor_tensor(
            prodc2[:], onehot2[:],
            iotaDb.unsqueeze(1).broadcast_to((128, CH, NCLS)), ALU.mult)
        cidm2 = pb.tile([128, CH], F32)
        nc.vector.tensor_reduce(cidm2[:], prodc2[:], AX, ALU.max)
        nc.vector.tensor_scalar(gall[:, :, 5], cidm2[:], -1.0, float(NCLS),
                                op0=ALU.mult, op1=ALU.add)
        nc.vector.tensor_copy(gall[:, :, 6], maxc2[:])
        nc.vector.tensor_copy(gall[:, :, 7], rfc[:])
        ngi = pb.tile([128, CH], F32)
        nc.vector.tensor_scalar_mul(ngi[:], rfc[:], -1.0)

        # class-specific delta via one-hot select
        dvw = gdall[:].rearrange("p k (c e) -> p k e c", c=NCLS, e=4)
        prod_dc = pb.tile([128, CH, 4, NCLS], F32)
        nc.vector.tensor_tensor(
            prod_dc[:], dvw,
            onehot2[:].unsqueeze(2).broadcast_to((128, CH, 4, NCLS)), ALU.mult)
        gd4 = pb.tile([128, CH, 4], F32)
        nc.vector.tensor_reduce(gd4[:], prod_dc[:], AX, ALU.add)

        # wave 1 replicate: (score, rowid) -> DRAM -> broadcast
        gT1 = pb.tile([2, NSLOT], F32)
        for k in range(CH):
            cs = CHS[k]
            nc.tensor.transpose(out=tr1_ps[0:2, 0:cs], in_=gall[0:cs, k, 6:8],
                                identity=identity[0:cs, 0:cs])
            nc.vector.tensor_copy(gT1[:, k * 128:k * 128 + cs],
                                  tr1_ps[0:2, 0:cs])
        w1d = pdr.tile([2, NSLOT], F32)
        nc.sync.dma_start(w1d[:], gT1[:])
        w1s = pb.tile([128, 2 * NSLOT], F32)
        nc.sync.dma_start(
            w1s[:], w1d[:].rearrange("a b -> (a b)").unsqueeze(0)
            .broadcast_to((128, 2 * NSLOT)))
        rep_s = w1s[:, 0:NSLOT]
        rep_gi = w1s[:, NSLOT:2 * NSLOT]

        # refine + clip
        dstd01 = pb.tile([128, CH, 2], F32)
        dstd23 = pb.tile([128, CH, 2], F32)
        nc.vector.tensor_scalar_mul(dstd01[:], gd4[:, :, 0:2], 0.1)
        nc.scalar.mul(dstd23[:], gd4[:, :, 2:4], 0.2)
        hwt = pb.tile([128, CH, 2], F32)
        nc.vector.tensor_tensor(hwt[:], gr2v[:, :, 2:4], gr2v[:, :, 0:2],
                                ALU.subtract)
        cyx = pb.tile([128, CH, 2], F32)
        nc.vector.scalar_tensor_tensor(cyx[:], hwt[:], 0.5, gr2v[:, :, 0:2],
                                       op0=ALU.mult, op1=ALU.add)
        dhw = pb.tile([128, CH, 2], F32)
        nc.vector.tensor_tensor(dhw[:], dstd01[:], hwt[:], ALU.mult)
        cyx2 = pb.tile([128, CH, 2], F32)
        nc.vector.tensor_tensor(cyx2[:], cyx[:], dhw[:], ALU.add)
        ehw = pb.tile([128, CH, 2], F32)
        nc.scalar.activation(ehw[:], dstd23[:], ACT.Exp)
        hw2 = pb.tile([128, CH, 2], F32)
        nc.vector.tensor_tensor(hw2[:], hwt[:], ehw[:], ALU.mult)
        yx1 = pb.tile([128, CH, 2], F32)
        yx2 = pb.tile([128, CH, 2], F32)
        nc.vector.scalar_tensor_tensor(yx1[:], hw2[:], -0.5, cyx2[:],
                                       op0=ALU.mult, op1=ALU.add)
        nc.vector.tensor_tensor(yx2[:], yx1[:], hw2[:], ALU.add)
        lo_b = winb[:, 0:2].unsqueeze(1).broadcast_to((128, CH, 2))
        hi_b = winb[:, 2:4].unsqueeze(1).broadcast_to((128, CH, 2))
        cl1 = pb.tile([128, CH, 2], F32)
        nc.vector.tensor_tensor(cl1[:], yx1[:], lo_b, ALU.max)
        nc.vector.tensor_tensor(gall[:, :, 0:2], cl1[:], hi_b, ALU.min)
        cl2 = pb.tile([128, CH, 2], F32)
        nc.vector.tensor_tensor(cl2[:], yx2[:], lo_b, ALU.max)
        nc.vector.tensor_tensor(gall[:, :, 2:4], cl2[:], hi_b, ALU.min)
        dyx = pb.tile([128, CH, 2], F32)
        nc.vector.tensor_tensor(dyx[:], gall[:, :, 2:4], gall[:, :, 0:2],
                                ALU.subtract)
        dyxr = pb.tile([128, CH, 2], F32)
        nc.vector.tensor_scalar_max(dyxr[:], dyx[:], 0.0)
        nc.vector.scalar_tensor_tensor(gall[:, :, 4], dyxr[:, :, 0], NMS_THR,
                                       dyxr[:, :, 1], op0=ALU.mult,
                                       op1=ALU.mult)
        # shifted-y pairwise views + ACT-bias preps
        nc.vector.scalar_tensor_tensor(gmat[:, :, 0], gall[:, :, 5], 2.0,
                                       gall[:, :, 0], op0=ALU.mult,
                                       op1=ALU.add)
        nc.vector.scalar_tensor_tensor(gmat[:, :, 2], gall[:, :, 5], 2.0,
                                       gall[:, :, 2], op0=ALU.mult,
                                       op1=ALU.add)
        nc.vector.tensor_copy(gmat[:, :, 1], gall[:, :, 1])
        nc.vector.tensor_copy(gmat[:, :, 3], gall[:, :, 3])
        nc.vector.tensor_copy(gmat[:, :, 4], gall[:, :, 4])
        ngm = pb.tile([128, CH, 2], F32)   # -(ys1, x1)
        nc.vector.tensor_scalar_mul(ngm[:], gmat[:, :, 0:2], -1.0)
        hw13 = pb.tile([128, CH, 2], F32)  # (1.3*h, w) for relu biases
        nc.vector.tensor_scalar(hw13[:, :, 0], dyxr[:, :, 0], 1.0 + NMS_THR,
                                None, op0=ALU.mult)
        nc.vector.tensor_copy(hw13[:, :, 1], dyxr[:, :, 1])
        # validity
        v1 = pb.tile([128, CH], F32)
        qv2 = pb.tile([128, CH], F32)
        nc.vector.tensor_scalar(v1[:], gall[:, :, 5], 1.0, None, op0=ALU.is_ge)
        nc.vector.tensor_tensor(qv2[:], v1[:], q2[:], ALU.mult)
        v2 = pb.tile([128, CH], F32)
        nc.vector.tensor_scalar(v2[:], maxc2[:], MIN_CONF, None, op0=ALU.is_ge)
        nc.vector.tensor_tensor(qv2[:], qv2[:], v2[:], ALU.mult)

        # order matrices: O = (2*sign(sc - rep_s) + sign(rep_gi - gic)) > 0
        O = []
        for k in range(CH):
            cp = CHS[k]
            g_t = pb.tile([128, NSLOT], F32, tag=f"g{k}")
            h_t = pb.tile([128, NSLOT], F32, tag=f"h{k}")
            nc.scalar.activation(g_t[0:cp, :], rep_s[0:cp, :], ACT.Sign,
                                 bias=gall[0:cp, k, 6:7], scale=-1.0)
            nc.scalar.activation(h_t[0:cp, :], rep_gi[0:cp, :], ACT.Sign,
                                 bias=ngi[0:cp, k:k + 1], scale=1.0)
            t2 = pb.tile([128, NSLOT], F32, tag=f"t2{k}")
            nc.vector.scalar_tensor_tensor(t2[0:cp, :], g_t[0:cp, :], 2.0,
                                           h_t[0:cp, :], op0=ALU.mult,
                                           op1=ALU.add)
            ok_t = pb.tile([128, NSLOT], BF16, tag=f"O{k}")
            nc.vector.tensor_scalar(ok_t[0:cp, :], t2[0:cp, :], 0.0, None,
                                    op0=ALU.is_gt)
            O.append(ok_t)

        # wave 2 replicate: (ys1 x1 ys2 x2 a03) -> DRAM -> broadcast
        gT2 = pb.tile([5, NSLOT], F32)
        for k in range(CH):
            cs = CHS[k]
            nc.tensor.transpose(out=tr2_ps[0:5, 0:cs], in_=gmat[0:cs, k, 0:5],
                                identity=identity[0:cs, 0:cs])
            nc.vector.tensor_copy(gT2[:, k * 128:k * 128 + cs],
                                  tr2_ps[0:5, 0:cs])
        w2d = pdr.tile([5, NSLOT], F32)
        nc.scalar.dma_start(w2d[:], gT2[:])
        w2s = pb.tile([128, 5 * NSLOT], F32)
        nc.scalar.dma_start(
            w2s[:], w2d[:].rearrange("a b -> (a b)").unsqueeze(0)
            .broadcast_to((128, 5 * NSLOT)))
        rep_ys1 = w2s[:, 0:NSLOT]
        rep_x1 = w2s[:, NSLOT:2 * NSLOT]
        rep_ys2 = w2s[:, 2 * NSLOT:3 * NSLOT]
        rep_x2 = w2s[:, 3 * NSLOT:4 * NSLOT]
        rep_a = w2s[:, 4 * NSLOT:5 * NSLOT]

        # suppression matrices: chunk0 DVE-style, chunk1 ACT-style
        S = []
        # chunk 0 (128 rows): DVE max/STT chain + one ACT relu
        cp = CHS[0]
        iy1 = pb.tile([128, NSLOT], F32, tag="iy10")
        ix1 = pb.tile([128, NSLOT], F32, tag="ix10")
        nc.vector.tensor_scalar_max(iy1[0:cp, :], rep_ys1[0:cp, :],
                                    gmat[0:cp, 0, 0:1])
        nc.vector.tensor_scalar_max(ix1[0:cp, :], rep_x1[0:cp, :],
                                    gmat[0:cp, 0, 1:2])
        dhp = pb.tile([128, NSLOT], F32, tag="dhp0")
        dwp = pb.tile([128, NSLOT], F32, tag="dwp0")
        nc.vector.scalar_tensor_tensor(dhp[0:cp, :], rep_ys2[0:cp, :],
                                       gmat[0:cp, 0, 2:3], iy1[0:cp, :],
                                       op0=ALU.min, op1=ALU.subtract)
        nc.vector.scalar_tensor_tensor(dwp[0:cp, :], rep_x2[0:cp, :],
                                       gmat[0:cp, 0, 3:4], ix1[0:cp, :],
                                       op0=ALU.min, op1=ALU.subtract)
        dh13 = pb.tile([128, NSLOT], F32, tag="dh130")
        nc.scalar.activation(dh13[0:cp, :], dhp[0:cp, :], ACT.Relu,
                             scale=1.0 + NMS_THR)
        inter13 = pb.tile([128, NSLOT], F32, tag="int0")
        nc.vector.scalar_tensor_tensor(inter13[0:cp, :], dwp[0:cp, :], 0.0,
                                       dh13[0:cp, :], op0=ALU.max,
                                       op1=ALU.mult)
        dmar = pb.tile([128, NSLOT], F32, tag="dmar0")
        nc.vector.scalar_tensor_tensor(dmar[0:cp, :], inter13[0:cp, :],
                                       gmat[0:cp, 0, 4:5], rep_a[0:cp, :],
                                       op0=ALU.subtract, op1=ALU.subtract)
        sk0 = pb.tile([128, NSLOT], BF16, tag="S0")
        nc.vector.scalar_tensor_tensor(sk0[0:cp, :], dmar[0:cp, :], 0.0,
                                       O[0][0:cp, :], op0=ALU.is_gt,
                                       op1=ALU.mult)
        S.append(sk0)
        # chunk 1 (64 rows): ACT relu-with-bias identities
        cp = CHS[1]
        r1 = pb.tile([128, NSLOT], F32, tag="r1")
        r2 = pb.tile([128, NSLOT], F32, tag="r2")
        r3 = pb.tile([128, NSLOT], F32, tag="r3")
        r4 = pb.tile([128, NSLOT], F32, tag="r4")
        nc.scalar.activation(r1[0:cp, :], rep_ys1[0:cp, :], ACT.Relu,
                             bias=ngm[0:cp, 1, 0:1], scale=1.0)
        nc.scalar.activation(r3[0:cp, :], rep_ys2[0:cp, :], ACT.Relu,
                             bias=gmat[0:cp, 1, 2:3], scale=-1.0)
        nc.scalar.activation(r2[0:cp, :], rep_x1[0:cp, :], ACT.Relu,
                             bias=ngm[0:cp, 1, 1:2], scale=1.0)
        nc.scalar.activation(r4[0:cp, :], rep_x2[0:cp, :], ACT.Relu,
                             bias=gmat[0:cp, 1, 3:4], scale=-1.0)
        r13 = pb.tile([128, NSLOT], F32, tag="r13")
        r24 = pb.tile([128, NSLOT], F32, tag="r24")
        nc.vector.tensor_tensor(r13[0:cp, :], r1[0:cp, :], r3[0:cp, :],
                                ALU.add)
        nc.vector.tensor_tensor(r24[0:cp, :], r2[0:cp, :], r4[0:cp, :],
                                ALU.add)
        dh13b = pb.tile([128, NSLOT], F32, tag="dh13b")
        dwrb = pb.tile([128, NSLOT], F32, tag="dwrb")
        nc.scalar.activation(dh13b[0:cp, :], r13[0:cp, :], ACT.Relu,
                             bias=hw13[0:cp, 1, 0:1], scale=-(1.0 + NMS_THR))
        nc.scalar.activation(dwrb[0:cp, :], r24[0:cp, :], ACT.Relu,
                             bias=hw13[0:cp, 1, 1:2], scale=-1.0)
        int1 = pb.tile([128, NSLOT], F32, tag="int1")
        nc.vector.tensor_tensor(int1[0:cp, :], dh13b[0:cp, :], dwrb[0:cp, :],
                                ALU.mult)
        dmar1 = pb.tile([128, NSLOT], F32, tag="dmar1")
        nc.vector.scalar_tensor_tensor(dmar1[0:cp, :], int1[0:cp, :],
                                       gmat[0:cp, 1, 4:5], rep_a[0:cp, :],
                                       op0=ALU.subtract, op1=ALU.subtract)
        sk1 = pb.tile([128, NSLOT], BF16, tag="S1")
        nc.vector.scalar_tensor_tensor(sk1[0:cp, :], dmar1[0:cp, :], 0.0,
                                       O[1][0:cp, :], op0=ALU.is_gt,
                                       op1=ALU.mult)
        S.append(sk1)

        # ---------------- phase 4: fixpoint + rank scatter -------------------
        kvA = pb.tile([128, CH], BF16)
        kvB = pb.tile([128, CH], BF16)
        nc.vector.memset(kvB[:], 0.0)
        nc.vector.tensor_copy(kvA[:], qv2[:])
        bufs = [kvA, kvB]
        for it in range(NITER):
            src = bufs[it % 2]
            dst = bufs[(it + 1) % 2]
            for kc in range(CH):
                cc = CHS[kc]
                sup_ps = pps.tile([128, 1], F32, tag="supps")
                for kp in range(CH):
                    nc.tensor.matmul(
                        sup_ps[0:cc, :],
                        S[kp][0:CHS[kp], kc * 128:kc * 128 + cc],
                        src[0:CHS[kp], kp:kp + 1],
                        start=(kp == 0), stop=(kp == CH - 1),
                    )
                nc.vector.scalar_tensor_tensor(dst[0:cc, kc:kc + 1],
                                               sup_ps[0:cc, :], 0.5,
                                               qv2[0:cc, kc:kc + 1],
                                               op0=ALU.is_lt, op1=ALU.mult)
        kept = bufs[NITER % 2]
        keptf = pb.tile([128, CH], F32)
        nc.vector.tensor_copy(keptf[:], kept[:])

        for kc in range(CH):
            cc = CHS[kc]
            rho_ps = pps.tile([128, 1], F32, tag="supps")
            for kp in range(CH):
                nc.tensor.matmul(
                    rho_ps[0:cc, :],
                    O[kp][0:CHS[kp], kc * 128:kc * 128 + cc],
                    kept[0:CHS[kp], kp:kp + 1],
                    start=(kp == 0), stop=(kp == CH - 1),
                )
            eqr = pb.tile([128, R], F32, tag=f"eqr{kc}")
            nc.vector.tensor_scalar(eqr[0:cc, :], iotaRf[0:cc, :],
                                    rho_ps[0:cc, 0:1], None, op0=ALU.is_equal)
            ohr = pb.tile([128, R], F32, tag=f"ohr{kc}")
            nc.vector.tensor_scalar_mul(ohr[0:cc, :], eqr[0:cc, :],
                                        keptf[0:cc, kc:kc + 1])
            nc.tensor.matmul(out_ps, ohr[0:cc, :], gall[0:cc, kc, :],
                             start=(kc == 0), stop=(kc == CH - 1))
        out_sb = pb.tile([R, 6], F32)
        nc.vector.tensor_copy(out_sb[:, 0:4], out_ps[0:R, 0:4])
        nc.vector.tensor_copy(out_sb[:, 4:6], out_ps[0:R, 5:7])
        nc.sync.dma_start(det[:], out_sb[:])

        if "dbg" in outs:
            d = outs["dbg"]
            nc.sync.dma_start(d["maxv"][:], maxv[:])
            nc.sync.dma_start(d["cnt32"][:], cnt32[:])
            nc.sync.dma_start(d["tstar"][:], tstar[:])
            nf_c = pb.tile([1, 1], F32)
            nc.vector.tensor_copy(nf_c[:], nf[:])
            nc.sync.dma_start(d["nf"][:], nf_c[:])
            nc.sync.dma_start(d["sgout"][:], sgout[:])
            nc.sync.dma_start(d["rfc"][:], rfc[:])
            nc.sync.dma_start(d["q2"][:], q2[:])
            nc.sync.dma_start(d["gall"][:],
                              gall[:].rearrange("p c e -> p (c e)"))
            nc.sync.dma_start(d["gmat"][:],
                              gmat[:].rearrange("p c e -> p (c e)"))
            nc.sync.dma_start(d["qv2"][:], qv2[:])
            keptd = pb.tile([128, CH], F32)
            nc.vector.tensor_copy(keptd[:], kept[:])
            nc.sync.dma_start(d["kept"][:], keptd[:])
            nc.sync.dma_start(d["w1s"][:], w1s[:])
            nc.sync.dma_start(d["w2s"][:], w2s[:])
            nc.sync.dma_start(d["gd4"][:],
                              gd4[:].rearrange("p c e -> p (c e)"))


_CACHE = {}


def _get_nc():
    if "nc" in _CACHE:
        return _CACHE["nc"]
    nc = bacc.Bacc("TRN2", target_bir_lowering=False, debug=False,
                   num_devices=NCORES)
    ins = {
        "joined": nc.dram_tensor("joined", [N, 4 + NCLS], F32,
                                 kind="ExternalInput").ap(),
        "ROIs": nc.dram_tensor("ROIs", [N, 4], F32, kind="ExternalInput").ap(),
        "probs": nc.dram_tensor("probs", [N, NCLS], F32,
                                kind="ExternalInput").ap(),
        "deltas": nc.dram_tensor("deltas", [N, NCLS, 4], F32,
                                 kind="ExternalInput").ap(),
        "window": nc.dram_tensor("window", [1, 4], F32,
                                 kind="ExternalInput").ap(),
    }
    outs = {
        "det": nc.dram_tensor("det", [R, 6], F32, kind="ExternalOutput").ap(),
    }
    with tile.TileContext(nc) as tc:
        build(nc, tc, outs, ins)
    nc.compile()
    _CACHE["nc"] = nc
    return nc


def make_in_maps(ROIs, probs, deltas, window):
    base = {
        "joined": np.ascontiguousarray(
            np.concatenate([np.asarray(ROIs, np.float32),
                            np.asarray(probs, np.float32)], axis=1)),
        "ROIs": np.ascontiguousarray(ROIs, dtype=np.float32),
        "probs": np.ascontiguousarray(probs, dtype=np.float32),
        "deltas": np.ascontiguousarray(deltas, dtype=np.float32),
        "window": np.ascontiguousarray(window, dtype=np.float32).reshape(1, 4),
    }
    return [dict(base) for _ in range(NCORES)]


def kernel(ROIs, probs, deltas, window, **kw):
    import concourse.bass_utils as bass_utils

    nc = _get_nc()
    res = bass_utils.run_bass_kernel_spmd(
        nc, make_in_maps(ROIs, probs, deltas, window),
        core_ids=list(range(NCORES)),
    )
    return np.asarray(res.results[0]["det"], dtype=np.float32)
